# revision 33
# baseline (speedup 1.0000x reference)
"""BERT-base (12L, C=768, H=12, T=512, V=32000) forward on 8 Trainium2 NeuronCores.

Strategy: data-parallel over batch (B=8 -> 1 batch element per core).
Per core everything is computed with transposed activations xT [C, T]
(channel-major, 6 partition-tiles of [128, 512]):
  - Weights are bf16 (stationary matmul operands; PE rate keys on the moving
    operand which stays f32r/bf16 at 1 cyc/row for free dims >= 256) ->
    halves the HBM weight traffic. Residual stream stays f32 (f32r).
  - softmax here is over the QUERY axis (reference softmax(dim=1) on
    [B,T,T]), so scores are built transposed: attT[k,q] = K @ Q^T (1/sqrt(D)
    folded into Wq on host) and the softmax is a free-axis softmax (ACT exp
    with accum_out row sums); normalization folds into V (scale rows 1/sum).
  - Engine balance: ACT = exp / relu / LN-affine / V+OC psum-copies,
    DVE = Q/K psum-copies, reciprocals, vs scales, residual adds, LN stats,
    Pool(gpsimd) = squares, partition broadcasts, LN elementwise (SBUF only).
  - Attention emission is staggered (V proj first, then per m-tile Q,K,
    scores, AV interleaved one pair behind) so ACT exp latency hides under
    PE matmuls.
  - Next-layer Wq/Wk/Wv/Wo tiles are DMA-prefetched interleaved into the FFN
    weight stream.
  - decoder: logits[t, v] = x @ dec_W (bf16) + dec_b, vocab in 64 chunks of
    500 columns; logits written bf16 and upcast on host.
Embedding gather + positional add run on host (0.01% of FLOPs).
"""

import sys, os

sys.path.insert(0, "/opt/trn_rl_repo")

import numpy as np

L, H, C, D, FF, V, T, B = 12, 12, 768, 64, 3072, 32000, 512, 8
NC = C // 128        # 6 channel tiles
NT = T // 128        # 4 token tiles
NFF = FF // 128      # 24 ffn tiles
VCW = 500            # vocab chunk width
VCN = V // VCW       # 64 vocab chunks
EPS = 1e-5
NCORES = 8

_ENGINE = {}


def _build_bass(n_layers=L, with_decoder=True, debug_xt=False):
    import concourse.bass as bass
    import concourse.mybir as mybir
    import concourse.tile as tile
    from concourse import bacc

    f32 = mybir.dt.float32
    f32r = mybir.dt.float32r
    bf16 = mybir.dt.bfloat16
    AF = mybir.ActivationFunctionType
    ALU = mybir.AluOpType

    nc = bacc.Bacc("TRN2", target_bir_lowering=False, debug=False,
                   num_devices=NCORES)

    # ---- DRAM I/O ----
    x0t_d = nc.dram_tensor("x0t", [C, T], f32, kind="ExternalInput").ap()
    wq_d = nc.dram_tensor("wq", [L, C, C], bf16, kind="ExternalInput").ap()
    wk_d = nc.dram_tensor("wk", [L, C, C], bf16, kind="ExternalInput").ap()
    wv_d = nc.dram_tensor("wv", [L, C, C], bf16, kind="ExternalInput").ap()
    wo_d = nc.dram_tensor("wo", [L, C, C], bf16, kind="ExternalInput").ap()
    w1_d = nc.dram_tensor("w1", [L, C, FF], bf16, kind="ExternalInput").ap()
    w2_d = nc.dram_tensor("w2", [L, FF, C], bf16, kind="ExternalInput").ap()
    # all per-layer param vectors, host-packed to [128, L, 6*NC+NFF]
    # (order: bo, b2, g1, be1, g2, be2 -- NC chunks each -- then b1 NFF)
    pv_d = nc.dram_tensor("pvec", [128, L, 6 * NC + NFF], f32,
                          kind="ExternalInput").ap()
    if with_decoder:
        decw_d = nc.dram_tensor("decw", [C, V], bf16, kind="ExternalInput").ap()
        decb_d = nc.dram_tensor("decb", [V], bf16, kind="ExternalInput").ap()
        out_d = nc.dram_tensor("logits", [T, V], bf16, kind="ExternalOutput").ap()
    if debug_xt:
        xt_o_d = nc.dram_tensor("xt_out", [C, T], f32, kind="ExternalOutput").ap()

    with tile.TileContext(nc) as tc:
        from contextlib import ExitStack

        with ExitStack() as octx:
            const = octx.enter_context(tc.tile_pool(name="const", bufs=1))
            xfp = octx.enter_context(tc.tile_pool(name="xfp", bufs=6))
            if with_decoder:
                dwp = octx.enter_context(tc.tile_pool(name="dwp", bufs=2))
                dbp = octx.enter_context(tc.tile_pool(name="dbp", bufs=3))
                dop = octx.enter_context(tc.tile_pool(name="dop", bufs=6))
            ctx = octx.enter_context(ExitStack())
            trunk = ctx.enter_context(tc.tile_pool(name="trunk", bufs=8))
            qkp = ctx.enter_context(tc.tile_pool(name="qkp", bufs=6))
            vvp = ctx.enter_context(tc.tile_pool(name="vvp", bufs=6))
            ocp = ctx.enter_context(tc.tile_pool(name="ocp", bufs=7))
            smp = ctx.enter_context(tc.tile_pool(name="smp", bufs=12))
            vsp = ctx.enter_context(tc.tile_pool(name="vsp", bufs=20))
            wqp = ctx.enter_context(tc.tile_pool(name="wqp", bufs=6))
            wkp = ctx.enter_context(tc.tile_pool(name="wkp", bufs=6))
            wvp = ctx.enter_context(tc.tile_pool(name="wvp", bufs=6))
            wop = ctx.enter_context(tc.tile_pool(name="wop", bufs=6))
            w1p = ctx.enter_context(tc.tile_pool(name="w1p", bufs=3))
            w2p = ctx.enter_context(tc.tile_pool(name="w2p", bufs=3))
            h1p = ctx.enter_context(tc.tile_pool(name="h1p", bufs=3))
            sqp = ctx.enter_context(tc.tile_pool(name="sqp", bufs=2))
            bcp = ctx.enter_context(tc.tile_pool(name="bcp", bufs=4))
            svp = ctx.enter_context(tc.tile_pool(name="svp", bufs=16))
            stp = ctx.enter_context(tc.tile_pool(name="stp", bufs=5))

            ones_mu = const.tile([128, 1], f32, name="ones_mu", tag="ones_mu")
            nc.vector.memset(ones_mu, -1.0 / C)
            ones_sq = const.tile([128, 1], f32, name="ones_sq", tag="ones_sq")
            nc.vector.memset(ones_sq, 1.0 / C)
            zerov = const.tile([128, 1], f32, name="zerov", tag="zerov")
            nc.vector.memset(zerov, 0.0)
            epsv = const.tile([1, 1], f32, name="epsv", tag="epsv")
            nc.vector.memset(epsv, EPS)

            # layer-0 input first in the DMA queue: it gates the first V-proj
            xT, xTb = [], []
            x0r = x0t_d.rearrange("(m p) t -> p m t", p=128)
            for m in range(NC):
                t = trunk.tile([128, T], f32r, name="xT", tag="xT", bufs=7)
                nc.sync.dma_start(out=t, in_=x0r[:, m, :].bitcast(f32r))
                xT.append(t)
                tb = trunk.tile([128, T], bf16, name="xTb", tag="xTb", bufs=7)
                nc.gpsimd.tensor_copy(tb, t)
                xTb.append(tb)

            W_POOLS = (wqp, wkp, wvp, wop)
            W_DRAMS = (wq_d, wk_d, wv_d, wo_d)

            def prefetch_weight(l, idx):
                mat, m = idx // NC, idx % NC
                t = W_POOLS[mat].tile([128, C], bf16, name="w", tag=f"w{mat}")
                r = W_DRAMS[mat][l].rearrange("(m p) n -> p m n", p=128)
                nc.sync.dma_start(out=t, in_=r[:, m, :])
                return mat, t

            def load_all_weights(l):
                # V-projection runs first in the layer body: load wv first
                cur = [[], [], [], []]
                for mat in (2, 0, 1, 3):
                    for m in range(NC):
                        _, t = prefetch_weight(l, mat * NC + m)
                        cur[mat].append(t)
                return cur

            cur = load_all_weights(0)

            # per-layer param vectors: one contiguous DMA, sliced below
            pv = const.tile([128, L, 6 * NC + NFF], f32, tag="pvec")
            nc.sync.dma_start(out=pv, in_=pv_d)
            bo_v = pv[:, :, 0 * NC:1 * NC]
            b2_v = pv[:, :, 1 * NC:2 * NC]
            g1_v = pv[:, :, 2 * NC:3 * NC]
            be1_v = pv[:, :, 3 * NC:4 * NC]
            g2_v = pv[:, :, 4 * NC:5 * NC]
            be2_v = pv[:, :, 5 * NC:6 * NC]
            b1_v = pv[:, :, 6 * NC:6 * NC + NFF]

            def layernorm(res, g_v, be_v, l):
                """res: NC [128,T] f32r tiles.

                Returns (out_f, out_b): out_f = (res - mu) * rstd * g  (f32r,
                WITHOUT +beta -- beta is folded into the next residual bias on
                host), out_b = out_f + beta (bf16, the matmul operand)."""
                with tc.tile_pool(name="ps_ln", bufs=2, space="PSUM") as psl:
                    ps_mu = psl.tile([1, T], f32, name="ln", tag="ln")
                    ps_sq = psl.tile([1, T], f32, name="ln", tag="ln")
                    for m in range(NC):
                        sq = sqp.tile([128, T], f32r, name="sq", tag="sq")
                        nc.gpsimd.tensor_mul(sq, res[m], res[m])
                        nc.tensor.matmul(ps_mu, ones_mu.bitcast(f32r), res[m],
                                         start=(m == 0), stop=(m == NC - 1))
                        nc.tensor.matmul(ps_sq, ones_sq.bitcast(f32r), sq,
                                         start=(m == 0), stop=(m == NC - 1))
                    # ps_mu = -mean ; ps_sq = E[x^2]
                    nmu = stp.tile([1, T], f32r, name="st", tag="st")
                    nc.vector.tensor_copy(nmu, ps_mu)
                    nmu_b = bcp.tile([128, T], f32r, name="bc", tag="bc")
                    nc.gpsimd.partition_broadcast(nmu_b, nmu)
                    mu2 = stp.tile([1, T], f32r, name="st", tag="st")
                    nc.vector.tensor_mul(mu2, nmu, nmu)
                    var = stp.tile([1, T], f32r, name="st", tag="st")
                    nc.vector.tensor_sub(var, ps_sq, mu2)
                    std = stp.tile([1, T], f32r, name="st", tag="st")
                    nc.scalar.activation(std, var, AF.Sqrt, bias=epsv[:, :],
                                         scale=1.0)
                    rstd = stp.tile([1, T], f32r, name="st", tag="st")
                    with nc.allow_low_precision(reason="f32r is fp32 storage"):
                        nc.vector.reciprocal(rstd, std)
                    rstd_b = bcp.tile([128, T], f32r, name="bc", tag="bc")
                    nc.gpsimd.partition_broadcast(rstd_b, rstd)
                out_f, out_b = [], []
                for m in range(NC):
                    # t0-add splits DVE/Pool; the scalar stt (TensorScalarPtr)
                    # is only a valid opcode on DVE.
                    eng = nc.vector if m < 4 else nc.gpsimd
                    t1 = trunk.tile([128, T], f32r, name="xln", tag="xln",
                                    bufs=7)
                    eng.tensor_add(t1, res[m], nmu_b)
                    nc.vector.scalar_tensor_tensor(
                        out=t1, in0=t1, scalar=g_v[:, l, m:m + 1],
                        in1=rstd_b, op0=ALU.mult, op1=ALU.mult)
                    tb = trunk.tile([128, T], bf16, name="xlnb", tag="xlnb",
                                    bufs=7)
                    nc.scalar.activation(tb, t1, AF.Identity,
                                         bias=be_v[:, l, m:m + 1],
                                         scale=1.0)
                    out_f.append(t1)
                    out_b.append(tb)
                return out_f, out_b

            for l in range(n_layers):
                wqt, wkt, wvt, wot = cur

                # ------------- V projection: Vt[t, c'] (bf16) -------------
                Vt = []
                with tc.tile_pool(name="ps_v", bufs=4, space="PSUM") as psv:
                    for tn in range(NT):
                        v = vvp.tile([128, C], bf16, name="vv", tag="vv")
                        for half in range(2):
                            pv = psv.tile([128, C // 2], f32, name="v", tag="v")
                            for ct in range(NC):
                                nc.tensor.matmul(
                                    pv, xTb[ct][:, tn * 128:(tn + 1) * 128],
                                    wvt[ct][:, half * 384:(half + 1) * 384],
                                    start=(ct == 0), stop=(ct == NC - 1))
                            nc.vector.tensor_copy(
                                v[:, half * 384:(half + 1) * 384], pv)
                        Vt.append(v)

                # ------- Q,K projections + scores + AV, staggered -------
                QT, KT = [None] * NC, [None] * NC
                SM = [[None] * NT for _ in range(H)]
                ISUM = [[None] * NT for _ in range(H)]
                OC = [ocp.tile([128, T], bf16, name="oc", tag="oc")
                      for _ in range(NC)]

                with tc.tile_pool(name="ps_qk", bufs=3, space="PSUM") as psqk, \
                     tc.tile_pool(name="ps_sc", bufs=4, space="PSUM") as pssc, \
                     tc.tile_pool(name="ps_o", bufs=1, space="PSUM") as pso:

                    def emit_proj(m, wt, store):
                        p = psqk.tile([128, T], f32, name="qk", tag="qk")
                        for ct in range(NC):
                            nc.tensor.matmul(p, wt[ct][:, m * 128:(m + 1) * 128],
                                             xTb[ct], start=(ct == 0),
                                             stop=(ct == NC - 1))
                        t = qkp.tile([128, T], bf16, name="qt", tag=store)
                        nc.vector.tensor_copy(t, p)
                        return t

                    def emit_scores(h):
                        hi, ho = h // 2, (h % 2) * 64
                        for kt in range(NT):
                            pa = pssc.tile([128, T], f32, name="att", tag="att")
                            nc.tensor.matmul(
                                pa,
                                KT[hi][ho:ho + 64, kt * 128:(kt + 1) * 128],
                                QT[hi][ho:ho + 64, :],
                                start=True, stop=True)
                            s = smp.tile([128, T], bf16, name="sm", tag="sm")
                            ss = svp.tile([128, 1], f32, name="ss", tag="ss")
                            nc.scalar.activation(s, pa, AF.Exp, bias=zerov[:, :],
                                                 scale=1.0, accum_out=ss)
                            iv = svp.tile([128, 1], f32, name="is", tag="is")
                            nc.vector.reciprocal(iv, ss)
                            vs = vsp.tile([128, 64], bf16, name="vs", tag="vs")
                            nc.vector.tensor_scalar_mul(
                                vs, Vt[kt][:, h * 64:(h + 1) * 64], iv)
                            SM[h][kt] = s
                            ISUM[h][kt] = vs

                    def emit_av(h):
                        hi, ho = h // 2, (h % 2) * 64
                        po = pso.tile([64, T], f32, name="oh", tag="oh")
                        for kt in range(NT):
                            nc.tensor.matmul(po, ISUM[h][kt], SM[h][kt],
                                             start=(kt == 0),
                                             stop=(kt == NT - 1))
                        nc.vector.tensor_copy(OC[hi][ho:ho + 64, :], po)

                    # staggered: AV trails scores by one m-pair
                    for m in range(NC):
                        QT[m] = emit_proj(m, wqt, "qt")
                        KT[m] = emit_proj(m, wkt, "kt")
                        if m > 0:
                            emit_av(2 * m - 2)
                        emit_scores(2 * m)
                        if m > 0:
                            emit_av(2 * m - 1)
                        emit_scores(2 * m + 1)
                    emit_av(2 * NC - 2)
                    emit_av(2 * NC - 1)

                # ---------------- out proj + residual + LN1 --------
                res1 = []
                with tc.tile_pool(name="ps_c", bufs=3, space="PSUM") as psc:
                    for m in range(NC):
                        py = psc.tile([128, T], f32, name="c", tag="c")
                        for ct in range(NC):
                            nc.tensor.matmul(py, wot[ct][:, m * 128:(m + 1) * 128],
                                             OC[ct], start=(ct == 0),
                                             stop=(ct == NC - 1))
                        r = trunk.tile([128, T], f32r, name="res", tag="res",
                                       bufs=7)
                        nc.vector.scalar_tensor_tensor(
                            out=r, in0=py.bitcast(f32r),
                            scalar=bo_v[:, l, m:m + 1], in1=xT[m],
                            op0=ALU.add, op1=ALU.add)
                        res1.append(r)
                xln, xlnb = layernorm(res1, g1_v, be1_v, l)

                # ---------------- FFN (+ next-layer weight prefetch) ------
                w1_r = w1_d[l]
                w2_r = w2_d[l].rearrange("(hh p) n -> p hh n", p=128)
                nxt = [[], [], [], []] if l + 1 < n_layers else None
                res2 = []
                with tc.tile_pool(name="ps_acc", bufs=6, space="PSUM") as psd, \
                     tc.tile_pool(name="ps_h1", bufs=2, space="PSUM") as psh:
                    acc = [psd.tile([128, T], f32, name="acc", tag="acc")
                           for _ in range(NC)]

                    def emit_w2(hh, w2t, h1):
                        for m in range(NC):
                            nc.tensor.matmul(acc[m], w2t[:, m * 128:(m + 1) * 128],
                                             h1, start=(hh == 0),
                                             stop=(hh == NFF - 1))

                    # W2 accumulation trails W1 by one hh so the PE never
                    # waits on the ACT relu of the current iteration.
                    pend = None
                    for hh in range(NFF):
                        w1t = w1p.tile([128, NC, 128], bf16, name="w1", tag="w1")
                        nc.sync.dma_start(
                            out=w1t,
                            in_=w1_r[:, hh * 128:(hh + 1) * 128]
                            .rearrange("(m p) n -> p m n", p=128))
                        w2t = w2p.tile([128, C], bf16, name="w2", tag="w2")
                        nc.sync.dma_start(out=w2t, in_=w2_r[:, hh, :])
                        if nxt is not None and hh < 4 * NC:
                            mat, t = prefetch_weight(l + 1, hh)
                            nxt[mat].append(t)
                        ph = psh.tile([128, T], f32, name="h1", tag="h1")
                        for ct in range(NC):
                            nc.tensor.matmul(ph, w1t[:, ct, :], xlnb[ct],
                                             start=(ct == 0), stop=(ct == NC - 1))
                        h1 = h1p.tile([128, T], bf16, name="h1s", tag="h1s")
                        nc.scalar.activation(h1, ph, AF.Relu,
                                             bias=b1_v[:, l, hh:hh + 1], scale=1.0)
                        if pend is not None:
                            emit_w2(*pend)
                        pend = (hh, w2t, h1)
                    emit_w2(*pend)
                    for m in range(NC):
                        r = trunk.tile([128, T], f32r, name="res", tag="res",
                                       bufs=7)
                        nc.vector.scalar_tensor_tensor(
                            out=r, in0=acc[m].bitcast(f32r),
                            scalar=b2_v[:, l, m:m + 1], in1=xln[m],
                            op0=ALU.add, op1=ALU.add)
                        res2.append(r)
                xT, xTb = layernorm(res2, g2_v, be2_v, l)
                if nxt is not None:
                    cur = nxt

            xf = []
            for m in range(NC):
                t = xfp.tile([128, T], bf16, name="xf", tag="xf")
                nc.gpsimd.tensor_copy(t, xTb[m])
                xf.append(t)
            if debug_xt:
                # xT carries the LN output without +beta (host folds beta
                # forward); add it back for the debug dump.
                xdbg = []
                for m in range(NC):
                    t = xfp.tile([128, T], f32, name="xfd", tag="xfd")
                    nc.scalar.activation(t, xT[m], AF.Identity,
                                         bias=be2_v[:, n_layers - 1, m:m + 1],
                                         scale=1.0)
                    xdbg.append(t)
            ctx.close()

            if debug_xt:
                xo_r = xt_o_d.rearrange("(m p) t -> p m t", p=128)
                for m in range(NC):
                    nc.sync.dma_start(out=xo_r[:, m, :], in_=xdbg[m])

            # ---------------- Decoder ----------------
            if with_decoder:
                with tc.tile_pool(name="ps_d", bufs=6, space="PSUM") as psd2:
                    for vc in range(VCN):
                        dwt = dwp.tile([128, NC, VCW], bf16, name="dw", tag="dw")
                        nc.sync.dma_start(
                            out=dwt,
                            in_=decw_d[:, vc * VCW:(vc + 1) * VCW]
                            .rearrange("(m p) v -> p m v", p=128))
                        db1 = dbp.tile([1, VCW], bf16, name="db1", tag="db1")
                        nc.sync.dma_start(
                            out=db1,
                            in_=decb_d[vc * VCW:(vc + 1) * VCW]
                            .rearrange("(a v) -> a v", a=1))
                        dbb = dbp.tile([128, VCW], bf16, name="dbb", tag="dbb")
                        nc.gpsimd.partition_broadcast(dbb, db1)
                        for tn in range(NT):
                            pd = psd2.tile([128, VCW], f32, name="d", tag="d")
                            for m in range(NC):
                                nc.tensor.matmul(
                                    pd, xf[m][:, tn * 128:(tn + 1) * 128],
                                    dwt[:, m, :], start=(m == 0),
                                    stop=(m == NC - 1))
                            ot = dop.tile([128, VCW], bf16, name="do", tag="do")
                            nc.vector.tensor_add(ot, pd, dbb)
                            nc.sync.dma_start(
                                out=out_d[tn * 128:(tn + 1) * 128,
                                          vc * VCW:(vc + 1) * VCW],
                                in_=ot)

    nc.compile()
    return nc


def _get_engine(n_layers=L, with_decoder=True, debug_xt=False):
    key = (n_layers, with_decoder, debug_xt)
    if key in _ENGINE:
        return _ENGINE[key]

    import jax
    import jax.numpy as jnp
    from jax.sharding import Mesh, PartitionSpec, NamedSharding
    from jax.experimental.shard_map import shard_map
    import concourse.mybir as mybir
    from concourse import bass2jax
    from concourse.bass2jax import _bass_exec_p, install_neuronx_cc_hook

    # Persistent NEFF cache: walrus compile of the full model takes tens of
    # minutes; key on the BIR bytes so identical builds reuse the binary.
    if not getattr(bass2jax, "_neff_cache_installed", False):
        import hashlib, shutil
        _orig_compile = bass2jax.compile_bir_kernel

        def _cached_compile(ant_bir_str, compile_dir_path, neff_name="file.neff"):
            cache_dir = os.path.expanduser("~/.cache/bass_neff")
            os.makedirs(cache_dir, exist_ok=True)
            key = hashlib.sha256(
                ant_bir_str if isinstance(ant_bir_str, bytes)
                else ant_bir_str.encode()).hexdigest()
            hit = os.path.join(cache_dir, f"{key}.neff")
            out = os.path.join(compile_dir_path, neff_name)
            if os.path.exists(hit):
                shutil.copyfile(hit, out)
                return out
            res = _orig_compile(ant_bir_str, compile_dir_path, neff_name)
            try:
                shutil.copyfile(res, hit)
            except OSError:
                pass
            return res

        bass2jax.compile_bir_kernel = _cached_compile
        bass2jax._neff_cache_installed = True

    install_neuronx_cc_hook()
    nc = _build_bass(n_layers, with_decoder, debug_xt)

    partition_name = (nc.partition_id_tensor.name
                      if nc.partition_id_tensor else None)
    in_names, out_names, out_avals = [], [], []
    zero_shapes = []
    for alloc in nc.m.functions[0].allocations:
        if not isinstance(alloc, mybir.MemoryLocationSet):
            continue
        name = alloc.memorylocations[0].name
        if alloc.kind == "ExternalInput":
            if name != partition_name:
                in_names.append(name)
        elif alloc.kind == "ExternalOutput":
            out_names.append(name)
            shape = tuple(alloc.tensor_shape)
            dtype = mybir.dt.np(alloc.dtype)
            out_avals.append(jax.core.ShapedArray(shape, dtype))
            zero_shapes.append((shape, dtype))
    n_params = len(in_names)
    all_in_names = in_names + out_names
    if partition_name is not None:
        all_in_names = all_in_names + [partition_name]

    def _body(*args):
        operands = list(args)
        if partition_name is not None:
            operands.append(bass2jax.partition_id_tensor())
        outs = _bass_exec_p.bind(
            *operands,
            out_avals=tuple(out_avals),
            in_names=tuple(all_in_names),
            out_names=tuple(out_names),
            lowering_input_output_aliases=(),
            sim_require_finite=True,
            sim_require_nnan=True,
            nc=nc,
        )
        return tuple(outs)

    devices = jax.devices()[:NCORES]
    mesh = Mesh(np.asarray(devices), ("core",))
    sharded_inputs = {"x0t"}
    in_specs = tuple(
        PartitionSpec("core") if n in sharded_inputs else PartitionSpec()
        for n in in_names) + (PartitionSpec("core"),) * len(out_names)
    out_specs = (PartitionSpec("core"),) * len(out_names)
    sharded = jax.jit(shard_map(_body, mesh=mesh, in_specs=in_specs,
                                out_specs=out_specs, check_rep=False),
                      keep_unused=True)

    shard = NamedSharding(mesh, PartitionSpec("core"))
    repl = NamedSharding(mesh, PartitionSpec())
    in_shardings = {n: (shard if n in sharded_inputs else repl)
                    for n in in_names}

    def make_zeros():
        return [
            jax.device_put(
                np.zeros((NCORES * s[0], *s[1:]), dt), shard)
            for (s, dt) in zero_shapes
        ]

    eng = dict(nc=nc, in_names=in_names, out_names=out_names,
               out_avals=out_avals, sharded=sharded, mesh=mesh, shard=shard,
               in_shardings=in_shardings,
               make_zeros=make_zeros, zeros=None, dev_args=None,
               dev_args_key=None)
    _ENGINE[key] = eng
    return eng


def _host_prep(inputs):
    """Returns dict name -> per-core-stacked array [NCORES*d0, ...]."""
    import ml_dtypes
    bf16 = ml_dtypes.bfloat16

    ids = np.asarray(inputs["input_ids"])
    emb = np.asarray(inputs["emb"], dtype=np.float32)
    pos = np.asarray(inputs["pos"], dtype=np.float32)
    x0 = emb[ids] + pos[None, :T]                      # [B, T, C]
    x0t = np.ascontiguousarray(x0.transpose(0, 2, 1))  # [B, C, T]

    Wq = np.asarray(inputs["Wq"], dtype=np.float32)
    Wk = np.asarray(inputs["Wk"], dtype=np.float32)
    Wv = np.asarray(inputs["Wv"], dtype=np.float32)
    # fold 1/sqrt(D) into Wq before the bf16 cast
    wq = np.ascontiguousarray(
        (Wq * 0.125).transpose(0, 2, 1, 3).reshape(L, C, C)).astype(bf16)
    wk = np.ascontiguousarray(
        Wk.transpose(0, 2, 1, 3).reshape(L, C, C)).astype(bf16)
    wv = np.ascontiguousarray(
        Wv.transpose(0, 2, 1, 3).reshape(L, C, C)).astype(bf16)

    def f32c(x):
        return np.ascontiguousarray(np.asarray(x, dtype=np.float32))

    def b16c(x):
        return np.ascontiguousarray(np.asarray(x, dtype=np.float32)).astype(bf16)

    # On device, layernorm() returns the f32r residual stream WITHOUT +beta;
    # compensate by folding beta into the next residual-add bias:
    #   res2 = (ffn + b2 + be1) + xln_t1   -> b2' = b2 + be1
    #   res1 = (attn + bo + be2[l-1]) + xT -> bo' = bo + shift(be2)
    be1 = f32c(inputs["ln1_b"])
    be2 = f32c(inputs["ln2_b"])
    b2f = f32c(inputs["b2"]) + be1
    bof = f32c(inputs["bo"]).copy()
    bof[1:] += be2[:-1]

    def chunkmaj(a, n):  # [L, n*128] -> [128, L, n]
        return np.transpose(a.reshape(L, n, 128), (2, 0, 1))

    pvec = np.concatenate([
        chunkmaj(bof, NC), chunkmaj(b2f, NC),
        chunkmaj(f32c(inputs["ln1_g"]), NC), chunkmaj(be1, NC),
        chunkmaj(f32c(inputs["ln2_g"]), NC), chunkmaj(be2, NC),
        chunkmaj(f32c(inputs["b1"]), NFF),
    ], axis=2)
    pvec = np.ascontiguousarray(pvec, dtype=np.float32)

    shared = {
        "wq": wq, "wk": wk, "wv": wv,
        "wo": b16c(inputs["Wo"]), "w1": b16c(inputs["W1"]),
        "w2": b16c(inputs["W2"]), "pvec": pvec,
        "decw": b16c(inputs["dec_W"]), "decb": b16c(inputs["dec_b"]),
    }
    stacked = {"x0t": x0t.reshape(B * C, T)}
    stacked.update(shared)
    return stacked


def _run(eng, stacked, want=None):
    import jax
    key = tuple(id(stacked[name]) for name in eng["in_names"])
    if eng["dev_args_key"] != key:
        eng["dev_args"] = [
            jax.device_put(stacked[name], eng["in_shardings"][name])
            for name in eng["in_names"]]
        eng["dev_args_key"] = key
    if eng["zeros"] is None:
        eng["zeros"] = eng["make_zeros"]()
    out = eng["sharded"](*eng["dev_args"], *eng["zeros"])
    res = {}
    for i, name in enumerate(eng["out_names"]):
        if want is not None and name not in want:
            continue
        a = np.asarray(out[i])
        res[name] = a.reshape(NCORES, -1, *a.shape[1:])
    return res


_PREP_CACHE = {}


def kernel(**inputs):
    eng = _get_engine()
    pkey = tuple(id(inputs[k]) for k in sorted(inputs))
    stacked = _PREP_CACHE.get(pkey)
    if stacked is None:
        stacked = _host_prep(inputs)
        _PREP_CACHE.clear()
        _PREP_CACHE[pkey] = stacked
    res = _run(eng, stacked, want=("logits",))
    logits = res["logits"].reshape(NCORES, T, V)
    return logits.astype(np.float32)


if __name__ == "__main__":
    rng = np.random.default_rng(0)
    dummy = {
        "input_ids": rng.integers(0, V, (B, T)),
        "emb": rng.standard_normal((V, C), dtype=np.float32) * 0.02,
        "pos": rng.standard_normal((T, C), dtype=np.float32) * 0.02,
        "Wq": rng.standard_normal((L, H, C, D), dtype=np.float32) * 0.02,
        "Wk": rng.standard_normal((L, H, C, D), dtype=np.float32) * 0.02,
        "Wv": rng.standard_normal((L, H, C, D), dtype=np.float32) * 0.02,
        "Wo": rng.standard_normal((L, C, C), dtype=np.float32) * 0.02,
        "bo": np.zeros((L, C), np.float32),
        "ln1_g": np.ones((L, C), np.float32),
        "ln1_b": np.zeros((L, C), np.float32),
        "W1": rng.standard_normal((L, C, FF), dtype=np.float32) * 0.02,
        "b1": np.zeros((L, FF), np.float32),
        "W2": rng.standard_normal((L, FF, C), dtype=np.float32) * 0.02,
        "b2": np.zeros((L, C), np.float32),
        "ln2_g": np.ones((L, C), np.float32),
        "ln2_b": np.zeros((L, C), np.float32),
        "dec_W": rng.standard_normal((C, V), dtype=np.float32) * 0.02,
        "dec_b": np.zeros((V,), np.float32),
    }
    out = kernel(**dummy)
    print("out", out.shape, out.dtype, float(np.abs(out).max()))


# revision 34
# speedup vs baseline: 1.2916x; 1.2916x over previous
"""BERT-base (12L, C=768, H=12, T=512, V=32000) forward on 8 Trainium2 NeuronCores.

Strategy: data-parallel over batch (B=8 -> 1 batch element per core).
Per core everything is computed with transposed activations xT [C, T]
(channel-major, 6 partition-tiles of [128, 512]):
  - Weights are bf16 (stationary matmul operands; PE rate keys on the moving
    operand which stays f32r/bf16 at 1 cyc/row for free dims >= 256) ->
    halves the HBM weight traffic. Residual stream stays f32 (f32r).
  - softmax here is over the QUERY axis (reference softmax(dim=1) on
    [B,T,T]), so scores are built transposed: attT[k,q] = K @ Q^T (1/sqrt(D)
    folded into Wq on host) and the softmax is a free-axis softmax (ACT exp
    with accum_out row sums); normalization folds into V (scale rows 1/sum).
  - Engine balance: ACT = exp / relu / LN-affine / V+OC psum-copies,
    DVE = Q/K psum-copies, reciprocals, vs scales, residual adds, LN stats,
    Pool(gpsimd) = squares, partition broadcasts, LN elementwise (SBUF only).
  - Attention emission is staggered (V proj first, then per m-tile Q,K,
    scores, AV interleaved one pair behind) so ACT exp latency hides under
    PE matmuls.
  - Next-layer Wq/Wk/Wv/Wo tiles are DMA-prefetched interleaved into the FFN
    weight stream.
  - decoder: logits[t, v] = x @ dec_W (bf16) + dec_b, vocab in 64 chunks of
    500 columns; logits written bf16 and upcast on host.
Embedding gather + positional add run on host (0.01% of FLOPs).
"""

import sys, os

sys.path.insert(0, "/opt/trn_rl_repo")

import numpy as np

L, H, C, D, FF, V, T, B = 12, 12, 768, 64, 3072, 32000, 512, 8
NC = C // 128        # 6 channel tiles
NT = T // 128        # 4 token tiles
NFF = FF // 128      # 24 ffn tiles
VCW = 500            # vocab chunk width
VCN = V // VCW       # 64 vocab chunks
EPS = 1e-5
NCORES = 8

_ENGINE = {}


def _build_bass(n_layers=L, with_decoder=True, debug_xt=False):
    import concourse.bass as bass
    import concourse.mybir as mybir
    import concourse.tile as tile
    from concourse import bacc

    f32 = mybir.dt.float32
    f32r = mybir.dt.float32r
    bf16 = mybir.dt.bfloat16
    AF = mybir.ActivationFunctionType
    ALU = mybir.AluOpType

    nc = bacc.Bacc("TRN2", target_bir_lowering=False, debug=False,
                   num_devices=NCORES)

    # ---- DRAM I/O ----
    x0t_d = nc.dram_tensor("x0t", [C, T], f32, kind="ExternalInput").ap()
    wq_d = nc.dram_tensor("wq", [L, C, C], bf16, kind="ExternalInput").ap()
    wk_d = nc.dram_tensor("wk", [L, C, C], bf16, kind="ExternalInput").ap()
    wv_d = nc.dram_tensor("wv", [L, C, C], bf16, kind="ExternalInput").ap()
    wo_d = nc.dram_tensor("wo", [L, C, C], bf16, kind="ExternalInput").ap()
    w1_d = nc.dram_tensor("w1", [L, C, FF], bf16, kind="ExternalInput").ap()
    w2_d = nc.dram_tensor("w2", [L, FF, C], bf16, kind="ExternalInput").ap()
    # all per-layer param vectors, host-packed to [128, L, 6*NC+NFF]
    # (order: bo, b2, g1, be1, g2, be2 -- NC chunks each -- then b1 NFF)
    pv_d = nc.dram_tensor("pvec", [128, L, 6 * NC + NFF], f32,
                          kind="ExternalInput").ap()
    if with_decoder:
        decw_d = nc.dram_tensor("decw", [C, V], bf16, kind="ExternalInput").ap()
        decb_d = nc.dram_tensor("decb", [V], bf16, kind="ExternalInput").ap()
        out_d = nc.dram_tensor("logits", [T, V], bf16, kind="ExternalOutput").ap()
    if debug_xt:
        xt_o_d = nc.dram_tensor("xt_out", [C, T], f32, kind="ExternalOutput").ap()

    with tile.TileContext(nc) as tc:
        from contextlib import ExitStack

        with ExitStack() as octx:
            const = octx.enter_context(tc.tile_pool(name="const", bufs=1))
            xfp = octx.enter_context(tc.tile_pool(name="xfp", bufs=6))
            if with_decoder:
                dwp = octx.enter_context(tc.tile_pool(name="dwp", bufs=2))
                dbp = octx.enter_context(tc.tile_pool(name="dbp", bufs=3))
                dop = octx.enter_context(tc.tile_pool(name="dop", bufs=6))
            ctx = octx.enter_context(ExitStack())
            trunk = ctx.enter_context(tc.tile_pool(name="trunk", bufs=8))
            qkp = ctx.enter_context(tc.tile_pool(name="qkp", bufs=6))
            vvp = ctx.enter_context(tc.tile_pool(name="vvp", bufs=6))
            ocp = ctx.enter_context(tc.tile_pool(name="ocp", bufs=7))
            smp = ctx.enter_context(tc.tile_pool(name="smp", bufs=12))
            vsp = ctx.enter_context(tc.tile_pool(name="vsp", bufs=20))
            wqp = ctx.enter_context(tc.tile_pool(name="wqp", bufs=6))
            wkp = ctx.enter_context(tc.tile_pool(name="wkp", bufs=6))
            wvp = ctx.enter_context(tc.tile_pool(name="wvp", bufs=6))
            wop = ctx.enter_context(tc.tile_pool(name="wop", bufs=6))
            w1p = ctx.enter_context(tc.tile_pool(name="w1p", bufs=3))
            w2p = ctx.enter_context(tc.tile_pool(name="w2p", bufs=3))
            h1p = ctx.enter_context(tc.tile_pool(name="h1p", bufs=3))
            sqp = ctx.enter_context(tc.tile_pool(name="sqp", bufs=2))
            bcp = ctx.enter_context(tc.tile_pool(name="bcp", bufs=4))
            svp = ctx.enter_context(tc.tile_pool(name="svp", bufs=16))
            stp = ctx.enter_context(tc.tile_pool(name="stp", bufs=5))

            ones_mu = const.tile([128, 1], f32, name="ones_mu", tag="ones_mu")
            nc.vector.memset(ones_mu, -1.0 / C)
            ones_sq = const.tile([128, 1], f32, name="ones_sq", tag="ones_sq")
            nc.vector.memset(ones_sq, 1.0 / C)
            zerov = const.tile([128, 1], f32, name="zerov", tag="zerov")
            nc.vector.memset(zerov, 0.0)
            epsv = const.tile([1, 1], f32, name="epsv", tag="epsv")
            nc.vector.memset(epsv, EPS)

            # layer-0 input first in the DMA queue: it gates the first V-proj
            xT, xTb = [], []
            x0r = x0t_d.rearrange("(m p) t -> p m t", p=128)
            for m in range(NC):
                t = trunk.tile([128, T], f32r, name="xT", tag="xT", bufs=7)
                nc.sync.dma_start(out=t, in_=x0r[:, m, :].bitcast(f32r))
                xT.append(t)
                tb = trunk.tile([128, T], bf16, name="xTb", tag="xTb", bufs=7)
                nc.gpsimd.tensor_copy(tb, t)
                xTb.append(tb)

            W_POOLS = (wqp, wkp, wvp, wop)
            W_DRAMS = (wq_d, wk_d, wv_d, wo_d)

            def prefetch_weight(l, idx):
                mat, m = idx // NC, idx % NC
                t = W_POOLS[mat].tile([128, C], bf16, name="w", tag=f"w{mat}")
                r = W_DRAMS[mat][l].rearrange("(m p) n -> p m n", p=128)
                nc.sync.dma_start(out=t, in_=r[:, m, :])
                return mat, t

            def load_all_weights(l):
                # V-projection runs first in the layer body: load wv first
                cur = [[], [], [], []]
                for mat in (2, 0, 1, 3):
                    for m in range(NC):
                        _, t = prefetch_weight(l, mat * NC + m)
                        cur[mat].append(t)
                return cur

            cur = load_all_weights(0)

            # per-layer param vectors: one contiguous DMA, sliced below
            pv = const.tile([128, L, 6 * NC + NFF], f32, tag="pvec")
            nc.sync.dma_start(out=pv, in_=pv_d)
            bo_v = pv[:, :, 0 * NC:1 * NC]
            b2_v = pv[:, :, 1 * NC:2 * NC]
            g1_v = pv[:, :, 2 * NC:3 * NC]
            be1_v = pv[:, :, 3 * NC:4 * NC]
            g2_v = pv[:, :, 4 * NC:5 * NC]
            be2_v = pv[:, :, 5 * NC:6 * NC]
            b1_v = pv[:, :, 6 * NC:6 * NC + NFF]

            def layernorm(res, g_v, be_v, l):
                """res: NC [128,T] f32r tiles.

                Returns (out_f, out_b): out_f = (res - mu) * rstd * g  (f32r,
                WITHOUT +beta -- beta is folded into the next residual bias on
                host), out_b = out_f + beta (bf16, the matmul operand)."""
                with tc.tile_pool(name="ps_ln", bufs=2, space="PSUM") as psl:
                    ps_mu = psl.tile([1, T], f32, name="ln", tag="ln")
                    ps_sq = psl.tile([1, T], f32, name="ln", tag="ln")
                    for m in range(NC):
                        sq = sqp.tile([128, T], f32r, name="sq", tag="sq")
                        nc.scalar.square(sq, res[m])
                        nc.tensor.matmul(ps_mu, ones_mu.bitcast(f32r), res[m],
                                         start=(m == 0), stop=(m == NC - 1))
                        nc.tensor.matmul(ps_sq, ones_sq.bitcast(f32r), sq,
                                         start=(m == 0), stop=(m == NC - 1))
                    # ps_mu = -mean ; ps_sq = E[x^2]
                    nmu = stp.tile([1, T], f32r, name="st", tag="st")
                    nc.vector.tensor_copy(nmu, ps_mu)
                    nmu_b = bcp.tile([128, T], f32r, name="bc", tag="bc")
                    nc.gpsimd.partition_broadcast(nmu_b, nmu)
                    mu2 = stp.tile([1, T], f32r, name="st", tag="st")
                    nc.vector.tensor_mul(mu2, nmu, nmu)
                    var = stp.tile([1, T], f32r, name="st", tag="st")
                    nc.vector.tensor_sub(var, ps_sq, mu2)
                    std = stp.tile([1, T], f32r, name="st", tag="st")
                    nc.scalar.activation(std, var, AF.Sqrt, bias=epsv[:, :],
                                         scale=1.0)
                    rstd = stp.tile([1, T], f32r, name="st", tag="st")
                    with nc.allow_low_precision(reason="f32r is fp32 storage"):
                        nc.vector.reciprocal(rstd, std)
                    rstd_b = bcp.tile([128, T], f32r, name="bc", tag="bc")
                    nc.gpsimd.partition_broadcast(rstd_b, rstd)
                out_f, out_b = [], []
                for m in range(NC):
                    # t0-add splits DVE/Pool; the scalar stt (TensorScalarPtr)
                    # is only a valid opcode on DVE.
                    eng = nc.vector if m < 4 else nc.gpsimd
                    t1 = trunk.tile([128, T], f32r, name="xln", tag="xln",
                                    bufs=7)
                    eng.tensor_add(t1, res[m], nmu_b)
                    nc.vector.scalar_tensor_tensor(
                        out=t1, in0=t1, scalar=g_v[:, l, m:m + 1],
                        in1=rstd_b, op0=ALU.mult, op1=ALU.mult)
                    tb = trunk.tile([128, T], bf16, name="xlnb", tag="xlnb",
                                    bufs=7)
                    nc.scalar.activation(tb, t1, AF.Identity,
                                         bias=be_v[:, l, m:m + 1],
                                         scale=1.0)
                    out_f.append(t1)
                    out_b.append(tb)
                return out_f, out_b

            for l in range(n_layers):
                wqt, wkt, wvt, wot = cur

                # ------------- V projection: Vt[t, c'] (bf16) -------------
                Vt = []
                with tc.tile_pool(name="ps_v", bufs=4, space="PSUM") as psv:
                    for tn in range(NT):
                        v = vvp.tile([128, C], bf16, name="vv", tag="vv")
                        for half in range(2):
                            pv = psv.tile([128, C // 2], f32, name="v", tag="v")
                            for ct in range(NC):
                                nc.tensor.matmul(
                                    pv, xTb[ct][:, tn * 128:(tn + 1) * 128],
                                    wvt[ct][:, half * 384:(half + 1) * 384],
                                    start=(ct == 0), stop=(ct == NC - 1))
                            nc.vector.tensor_copy(
                                v[:, half * 384:(half + 1) * 384], pv)
                        Vt.append(v)

                # ------- Q,K projections + scores + AV, staggered -------
                QT, KT = [None] * NC, [None] * NC
                SM = [[None] * NT for _ in range(H)]
                ISUM = [[None] * NT for _ in range(H)]
                OC = [ocp.tile([128, T], bf16, name="oc", tag="oc")
                      for _ in range(NC)]

                with tc.tile_pool(name="ps_qk", bufs=3, space="PSUM") as psqk, \
                     tc.tile_pool(name="ps_sc", bufs=4, space="PSUM") as pssc, \
                     tc.tile_pool(name="ps_o", bufs=1, space="PSUM") as pso:

                    def emit_proj(m, wt, store):
                        p = psqk.tile([128, T], f32, name="qk", tag="qk")
                        for ct in range(NC):
                            nc.tensor.matmul(p, wt[ct][:, m * 128:(m + 1) * 128],
                                             xTb[ct], start=(ct == 0),
                                             stop=(ct == NC - 1))
                        t = qkp.tile([128, T], bf16, name="qt", tag=store)
                        nc.vector.tensor_copy(t, p)
                        return t

                    def emit_scores(h):
                        hi, ho = h // 2, (h % 2) * 64
                        for kt in range(NT):
                            pa = pssc.tile([128, T], f32, name="att", tag="att")
                            nc.tensor.matmul(
                                pa,
                                KT[hi][ho:ho + 64, kt * 128:(kt + 1) * 128],
                                QT[hi][ho:ho + 64, :],
                                start=True, stop=True)
                            s = smp.tile([128, T], bf16, name="sm", tag="sm")
                            ss = svp.tile([128, 1], f32, name="ss", tag="ss")
                            nc.scalar.activation(s, pa, AF.Exp, bias=zerov[:, :],
                                                 scale=1.0, accum_out=ss)
                            iv = svp.tile([128, 1], f32, name="is", tag="is")
                            nc.vector.reciprocal(iv, ss)
                            vs = vsp.tile([128, 64], bf16, name="vs", tag="vs")
                            nc.vector.tensor_scalar_mul(
                                vs, Vt[kt][:, h * 64:(h + 1) * 64], iv)
                            SM[h][kt] = s
                            ISUM[h][kt] = vs

                    def emit_av(h):
                        hi, ho = h // 2, (h % 2) * 64
                        po = pso.tile([64, T], f32, name="oh", tag="oh")
                        for kt in range(NT):
                            nc.tensor.matmul(po, ISUM[h][kt], SM[h][kt],
                                             start=(kt == 0),
                                             stop=(kt == NT - 1))
                        nc.vector.tensor_copy(OC[hi][ho:ho + 64, :], po)

                    # staggered: AV trails scores by one m-pair
                    for m in range(NC):
                        QT[m] = emit_proj(m, wqt, "qt")
                        KT[m] = emit_proj(m, wkt, "kt")
                        if m > 0:
                            emit_av(2 * m - 2)
                        emit_scores(2 * m)
                        if m > 0:
                            emit_av(2 * m - 1)
                        emit_scores(2 * m + 1)
                    emit_av(2 * NC - 2)
                    emit_av(2 * NC - 1)

                # ---------------- out proj + residual + LN1 --------
                res1 = []
                with tc.tile_pool(name="ps_c", bufs=3, space="PSUM") as psc:
                    for m in range(NC):
                        py = psc.tile([128, T], f32, name="c", tag="c")
                        for ct in range(NC):
                            nc.tensor.matmul(py, wot[ct][:, m * 128:(m + 1) * 128],
                                             OC[ct], start=(ct == 0),
                                             stop=(ct == NC - 1))
                        r = trunk.tile([128, T], f32r, name="res", tag="res",
                                       bufs=7)
                        nc.vector.scalar_tensor_tensor(
                            out=r, in0=py.bitcast(f32r),
                            scalar=bo_v[:, l, m:m + 1], in1=xT[m],
                            op0=ALU.add, op1=ALU.add)
                        res1.append(r)
                xln, xlnb = layernorm(res1, g1_v, be1_v, l)

                # ---------------- FFN (+ next-layer weight prefetch) ------
                w1_r = w1_d[l]
                w2_r = w2_d[l].rearrange("(hh p) n -> p hh n", p=128)
                nxt = [[], [], [], []] if l + 1 < n_layers else None
                res2 = []
                with tc.tile_pool(name="ps_acc", bufs=6, space="PSUM") as psd, \
                     tc.tile_pool(name="ps_h1", bufs=2, space="PSUM") as psh:
                    acc = [psd.tile([128, T], f32, name="acc", tag="acc")
                           for _ in range(NC)]

                    def emit_w2(hh, w2t, h1):
                        for m in range(NC):
                            nc.tensor.matmul(acc[m], w2t[:, m * 128:(m + 1) * 128],
                                             h1, start=(hh == 0),
                                             stop=(hh == NFF - 1))

                    # W2 accumulation trails W1 by one hh so the PE never
                    # waits on the ACT relu of the current iteration.
                    pend = None
                    for hh in range(NFF):
                        w1t = w1p.tile([128, NC, 128], bf16, name="w1", tag="w1")
                        nc.sync.dma_start(
                            out=w1t,
                            in_=w1_r[:, hh * 128:(hh + 1) * 128]
                            .rearrange("(m p) n -> p m n", p=128))
                        w2t = w2p.tile([128, C], bf16, name="w2", tag="w2")
                        nc.sync.dma_start(out=w2t, in_=w2_r[:, hh, :])
                        if nxt is not None and hh < 4 * NC:
                            mat, t = prefetch_weight(l + 1, hh)
                            nxt[mat].append(t)
                        ph = psh.tile([128, T], f32, name="h1", tag="h1")
                        for ct in range(NC):
                            nc.tensor.matmul(ph, w1t[:, ct, :], xlnb[ct],
                                             start=(ct == 0), stop=(ct == NC - 1))
                        h1 = h1p.tile([128, T], bf16, name="h1s", tag="h1s")
                        nc.scalar.activation(h1, ph, AF.Relu,
                                             bias=b1_v[:, l, hh:hh + 1], scale=1.0)
                        if pend is not None:
                            emit_w2(*pend)
                        pend = (hh, w2t, h1)
                    emit_w2(*pend)
                    for m in range(NC):
                        r = trunk.tile([128, T], f32r, name="res", tag="res",
                                       bufs=7)
                        nc.vector.scalar_tensor_tensor(
                            out=r, in0=acc[m].bitcast(f32r),
                            scalar=b2_v[:, l, m:m + 1], in1=xln[m],
                            op0=ALU.add, op1=ALU.add)
                        res2.append(r)
                xT, xTb = layernorm(res2, g2_v, be2_v, l)
                if nxt is not None:
                    cur = nxt

            xf = xTb
            if debug_xt:
                # xT carries the LN output without +beta (host folds beta
                # forward); add it back for the debug dump.
                xdbg = []
                for m in range(NC):
                    t = xfp.tile([128, T], f32, name="xfd", tag="xfd")
                    nc.scalar.activation(t, xT[m], AF.Identity,
                                         bias=be2_v[:, n_layers - 1, m:m + 1],
                                         scale=1.0)
                    xdbg.append(t)

            if debug_xt:
                xo_r = xt_o_d.rearrange("(m p) t -> p m t", p=128)
                for m in range(NC):
                    nc.sync.dma_start(out=xo_r[:, m, :], in_=xdbg[m])

            # ---------------- Decoder ----------------
            if with_decoder:
                with tc.tile_pool(name="ps_d", bufs=6, space="PSUM") as psd2:
                    for vc in range(VCN):
                        dwt = dwp.tile([128, NC, VCW], bf16, name="dw", tag="dw")
                        nc.sync.dma_start(
                            out=dwt,
                            in_=decw_d[:, vc * VCW:(vc + 1) * VCW]
                            .rearrange("(m p) v -> p m v", p=128))
                        db1 = dbp.tile([1, VCW], bf16, name="db1", tag="db1")
                        nc.sync.dma_start(
                            out=db1,
                            in_=decb_d[vc * VCW:(vc + 1) * VCW]
                            .rearrange("(a v) -> a v", a=1))
                        dbb = dbp.tile([128, VCW], bf16, name="dbb", tag="dbb")
                        nc.gpsimd.partition_broadcast(dbb, db1)
                        for tn in range(NT):
                            pd = psd2.tile([128, VCW], f32, name="d", tag="d")
                            for m in range(NC):
                                nc.tensor.matmul(
                                    pd, xf[m][:, tn * 128:(tn + 1) * 128],
                                    dwt[:, m, :], start=(m == 0),
                                    stop=(m == NC - 1))
                            ot = dop.tile([128, VCW], bf16, name="do", tag="do")
                            nc.vector.tensor_add(ot, pd, dbb)
                            nc.sync.dma_start(
                                out=out_d[tn * 128:(tn + 1) * 128,
                                          vc * VCW:(vc + 1) * VCW],
                                in_=ot)

    nc.compile()
    return nc


def _get_engine(n_layers=L, with_decoder=True, debug_xt=False):
    key = (n_layers, with_decoder, debug_xt)
    if key in _ENGINE:
        return _ENGINE[key]

    import jax
    import jax.numpy as jnp
    from jax.sharding import Mesh, PartitionSpec, NamedSharding
    from jax.experimental.shard_map import shard_map
    import concourse.mybir as mybir
    from concourse import bass2jax
    from concourse.bass2jax import _bass_exec_p, install_neuronx_cc_hook

    # Persistent NEFF cache: walrus compile of the full model takes tens of
    # minutes; key on the BIR bytes so identical builds reuse the binary.
    if not getattr(bass2jax, "_neff_cache_installed", False):
        import hashlib, shutil
        _orig_compile = bass2jax.compile_bir_kernel

        def _cached_compile(ant_bir_str, compile_dir_path, neff_name="file.neff"):
            cache_dir = os.path.expanduser("~/.cache/bass_neff")
            os.makedirs(cache_dir, exist_ok=True)
            key = hashlib.sha256(
                ant_bir_str if isinstance(ant_bir_str, bytes)
                else ant_bir_str.encode()).hexdigest()
            hit = os.path.join(cache_dir, f"{key}.neff")
            out = os.path.join(compile_dir_path, neff_name)
            if os.path.exists(hit):
                shutil.copyfile(hit, out)
                return out
            res = _orig_compile(ant_bir_str, compile_dir_path, neff_name)
            try:
                shutil.copyfile(res, hit)
            except OSError:
                pass
            return res

        bass2jax.compile_bir_kernel = _cached_compile
        bass2jax._neff_cache_installed = True

    install_neuronx_cc_hook()
    nc = _build_bass(n_layers, with_decoder, debug_xt)

    partition_name = (nc.partition_id_tensor.name
                      if nc.partition_id_tensor else None)
    in_names, out_names, out_avals = [], [], []
    zero_shapes = []
    for alloc in nc.m.functions[0].allocations:
        if not isinstance(alloc, mybir.MemoryLocationSet):
            continue
        name = alloc.memorylocations[0].name
        if alloc.kind == "ExternalInput":
            if name != partition_name:
                in_names.append(name)
        elif alloc.kind == "ExternalOutput":
            out_names.append(name)
            shape = tuple(alloc.tensor_shape)
            dtype = mybir.dt.np(alloc.dtype)
            out_avals.append(jax.core.ShapedArray(shape, dtype))
            zero_shapes.append((shape, dtype))
    n_params = len(in_names)
    all_in_names = in_names + out_names
    if partition_name is not None:
        all_in_names = all_in_names + [partition_name]

    def _body(*args):
        operands = list(args)
        if partition_name is not None:
            operands.append(bass2jax.partition_id_tensor())
        outs = _bass_exec_p.bind(
            *operands,
            out_avals=tuple(out_avals),
            in_names=tuple(all_in_names),
            out_names=tuple(out_names),
            lowering_input_output_aliases=(),
            sim_require_finite=True,
            sim_require_nnan=True,
            nc=nc,
        )
        return tuple(outs)

    devices = jax.devices()[:NCORES]
    mesh = Mesh(np.asarray(devices), ("core",))
    sharded_inputs = {"x0t"}
    in_specs = tuple(
        PartitionSpec("core") if n in sharded_inputs else PartitionSpec()
        for n in in_names) + (PartitionSpec("core"),) * len(out_names)
    out_specs = (PartitionSpec("core"),) * len(out_names)
    sharded = jax.jit(shard_map(_body, mesh=mesh, in_specs=in_specs,
                                out_specs=out_specs, check_rep=False),
                      keep_unused=True)

    shard = NamedSharding(mesh, PartitionSpec("core"))
    repl = NamedSharding(mesh, PartitionSpec())
    in_shardings = {n: (shard if n in sharded_inputs else repl)
                    for n in in_names}

    def make_zeros():
        return [
            jax.device_put(
                np.zeros((NCORES * s[0], *s[1:]), dt), shard)
            for (s, dt) in zero_shapes
        ]

    eng = dict(nc=nc, in_names=in_names, out_names=out_names,
               out_avals=out_avals, sharded=sharded, mesh=mesh, shard=shard,
               in_shardings=in_shardings,
               make_zeros=make_zeros, zeros=None, dev_args=None,
               dev_args_key=None)
    _ENGINE[key] = eng
    return eng


def _host_prep(inputs):
    """Returns dict name -> per-core-stacked array [NCORES*d0, ...]."""
    import ml_dtypes
    bf16 = ml_dtypes.bfloat16

    ids = np.asarray(inputs["input_ids"])
    emb = np.asarray(inputs["emb"], dtype=np.float32)
    pos = np.asarray(inputs["pos"], dtype=np.float32)
    x0 = emb[ids] + pos[None, :T]                      # [B, T, C]
    x0t = np.ascontiguousarray(x0.transpose(0, 2, 1))  # [B, C, T]

    Wq = np.asarray(inputs["Wq"], dtype=np.float32)
    Wk = np.asarray(inputs["Wk"], dtype=np.float32)
    Wv = np.asarray(inputs["Wv"], dtype=np.float32)
    # fold 1/sqrt(D) into Wq before the bf16 cast
    wq = np.ascontiguousarray(
        (Wq * 0.125).transpose(0, 2, 1, 3).reshape(L, C, C)).astype(bf16)
    wk = np.ascontiguousarray(
        Wk.transpose(0, 2, 1, 3).reshape(L, C, C)).astype(bf16)
    wv = np.ascontiguousarray(
        Wv.transpose(0, 2, 1, 3).reshape(L, C, C)).astype(bf16)

    def f32c(x):
        return np.ascontiguousarray(np.asarray(x, dtype=np.float32))

    def b16c(x):
        return np.ascontiguousarray(np.asarray(x, dtype=np.float32)).astype(bf16)

    # On device, layernorm() returns the f32r residual stream WITHOUT +beta;
    # compensate by folding beta into the next residual-add bias:
    #   res2 = (ffn + b2 + be1) + xln_t1   -> b2' = b2 + be1
    #   res1 = (attn + bo + be2[l-1]) + xT -> bo' = bo + shift(be2)
    be1 = f32c(inputs["ln1_b"])
    be2 = f32c(inputs["ln2_b"])
    b2f = f32c(inputs["b2"]) + be1
    bof = f32c(inputs["bo"]).copy()
    bof[1:] += be2[:-1]

    def chunkmaj(a, n):  # [L, n*128] -> [128, L, n]
        return np.transpose(a.reshape(L, n, 128), (2, 0, 1))

    pvec = np.concatenate([
        chunkmaj(bof, NC), chunkmaj(b2f, NC),
        chunkmaj(f32c(inputs["ln1_g"]), NC), chunkmaj(be1, NC),
        chunkmaj(f32c(inputs["ln2_g"]), NC), chunkmaj(be2, NC),
        chunkmaj(f32c(inputs["b1"]), NFF),
    ], axis=2)
    pvec = np.ascontiguousarray(pvec, dtype=np.float32)

    shared = {
        "wq": wq, "wk": wk, "wv": wv,
        "wo": b16c(inputs["Wo"]), "w1": b16c(inputs["W1"]),
        "w2": b16c(inputs["W2"]), "pvec": pvec,
        "decw": b16c(inputs["dec_W"]), "decb": b16c(inputs["dec_b"]),
    }
    stacked = {"x0t": x0t.reshape(B * C, T)}
    stacked.update(shared)
    return stacked


def _run(eng, stacked, want=None):
    import jax
    key = tuple(id(stacked[name]) for name in eng["in_names"])
    if eng["dev_args_key"] != key:
        eng["dev_args"] = [
            jax.device_put(stacked[name], eng["in_shardings"][name])
            for name in eng["in_names"]]
        eng["dev_args_key"] = key
    if eng["zeros"] is None:
        eng["zeros"] = eng["make_zeros"]()
    out = eng["sharded"](*eng["dev_args"], *eng["zeros"])
    res = {}
    for i, name in enumerate(eng["out_names"]):
        if want is not None and name not in want:
            continue
        a = np.asarray(out[i])
        res[name] = a.reshape(NCORES, -1, *a.shape[1:])
    return res


_PREP_CACHE = {}


def kernel(**inputs):
    eng = _get_engine()
    pkey = tuple(id(inputs[k]) for k in sorted(inputs))
    stacked = _PREP_CACHE.get(pkey)
    if stacked is None:
        stacked = _host_prep(inputs)
        _PREP_CACHE.clear()
        _PREP_CACHE[pkey] = stacked
    res = _run(eng, stacked, want=("logits",))
    logits = res["logits"].reshape(NCORES, T, V)
    return logits.astype(np.float32)


if __name__ == "__main__":
    rng = np.random.default_rng(0)
    dummy = {
        "input_ids": rng.integers(0, V, (B, T)),
        "emb": rng.standard_normal((V, C), dtype=np.float32) * 0.02,
        "pos": rng.standard_normal((T, C), dtype=np.float32) * 0.02,
        "Wq": rng.standard_normal((L, H, C, D), dtype=np.float32) * 0.02,
        "Wk": rng.standard_normal((L, H, C, D), dtype=np.float32) * 0.02,
        "Wv": rng.standard_normal((L, H, C, D), dtype=np.float32) * 0.02,
        "Wo": rng.standard_normal((L, C, C), dtype=np.float32) * 0.02,
        "bo": np.zeros((L, C), np.float32),
        "ln1_g": np.ones((L, C), np.float32),
        "ln1_b": np.zeros((L, C), np.float32),
        "W1": rng.standard_normal((L, C, FF), dtype=np.float32) * 0.02,
        "b1": np.zeros((L, FF), np.float32),
        "W2": rng.standard_normal((L, FF, C), dtype=np.float32) * 0.02,
        "b2": np.zeros((L, C), np.float32),
        "ln2_g": np.ones((L, C), np.float32),
        "ln2_b": np.zeros((L, C), np.float32),
        "dec_W": rng.standard_normal((C, V), dtype=np.float32) * 0.02,
        "dec_b": np.zeros((V,), np.float32),
    }
    out = kernel(**dummy)
    print("out", out.shape, out.dtype, float(np.abs(out).max()))
